# revision 23
# baseline (speedup 1.0000x reference)
"""GuidedFilter (r=15, eps=0.5) Trainium2 Bass kernel.

Full inputs: guide, input_map [16,1,1024,1024] f32. Data-parallel over 8
NeuronCores (2 images/core). Per image:
  box(x) = Vpass(Hpass(x)) with 31-tap window sums, reflect padding.
  - H direction (free axis): tensor_tensor_scan per row-tile, split across
    DVE and Pool (gpsimd) engines to balance load.
  - V direction (partition axis): PE band matmuls with constant bf16
    weights (reflect folded into band blocks, 1/961 normalization folded
    into the weights), fp32 PSUM accumulate.
  - eps is folded into the II H-scan init (+31*EPS offsets every window
    sum, which the normalized V-pass turns into +EPS on mean_II).
  - a = cov * r, r = 1/(var+eps) via Act-engine Abs_reciprocal_sqrt
    squared on DVE; elementwise chain mostly bf16.
"""

import numpy as np
import ml_dtypes

R = 15
K = 2 * R + 1  # 31
EPS = 0.5
NORM = 1.0 / (K * K)  # 1/961

_CACHE = {}


def _build_band_weights(Hc, NT):
    """Wf[k, m] = NORM * weight of input row k in output row m's reflect window."""
    Wf = np.zeros((Hc, Hc), np.float32)
    for m in range(Hc):
        for t in range(m - R, m + R + 1):
            k = t
            if k < 0:
                k = -k
            if k > Hc - 1:
                k = 2 * (Hc - 1) - k
            Wf[k, m] += 1.0
    Wf *= NORM
    # Pack per out-tile j into [128, 3*128]:
    #   cols 0:128   = center block  (in-tile j)
    #   cols 128:256 = top edge      (in-tile j-1 rows 113:128 -> rows 64:128 window)
    #   cols 256:384 = bottom edge   (in-tile j+1 rows 0:15)
    wv = np.zeros((NT, 128, 384), np.float32)
    for j in range(NT):
        r0 = j * 128
        wv[j, :, 0:128] = Wf[r0 : r0 + 128, r0 : r0 + 128]
        if j > 0:
            wv[j, 64:128, 128:256] = Wf[r0 - 64 : r0, r0 : r0 + 128]
        if j < NT - 1:
            wv[j, 0:15, 256:384] = Wf[r0 + 128 : r0 + 143, r0 : r0 + 128]
    return wv.astype(ml_dtypes.bfloat16)


def build_nc(n_img, Hc, Wc):
    """Build the Bass module for one core processing n_img images of [Hc, Wc]."""
    import concourse.bass as bass
    import concourse.tile as tile
    from concourse import bacc, mybir

    P = 128
    NT = Hc // P
    PW = Wc + 32          # padded width; interior at cols 16..16+Wc
    CH = min(512, Wc)     # psum chunk width
    NC_ = Wc // CH        # chunks per tile
    f32 = mybir.dt.float32
    bf16 = mybir.dt.bfloat16
    AX = mybir.AxisListType.X
    OP = mybir.AluOpType
    AF = mybir.ActivationFunctionType

    nc = bacc.Bacc("TRN2", target_bir_lowering=False, debug=False)
    g_dram = nc.dram_tensor("guide", [n_img, Hc, Wc], f32, kind="ExternalInput")
    p_dram = nc.dram_tensor("input_map", [n_img, Hc, Wc], f32, kind="ExternalInput")
    wv_dram = nc.dram_tensor("wv", [NT, 128, 384], bf16, kind="ExternalInput")
    o_dram = nc.dram_tensor("out", [n_img, Hc, Wc], f32, kind="ExternalOutput")
    gap, pap, wap, oap = g_dram.ap(), p_dram.ap(), wv_dram.ap(), o_dram.ap()

    with tile.TileContext(nc) as tc:
        wpool = tc.alloc_tile_pool(name="wv", bufs=1)
        wv_sb = []
        for j in range(NT):
            wt = wpool.tile([128, 384], bf16, tag=f"wv{j}", name=f"wv{j}")
            nc.sync.dma_start(wt[:], wap[j])
            wv_sb.append(wt)

        xpi_pool = tc.alloc_tile_pool(name="xpi", bufs=NT + 1)   # guide, image-long
        xpp_pool = tc.alloc_tile_pool(name="xpp", bufs=3)
        xpm_pool = tc.alloc_tile_pool(name="xpm", bufs=2)        # Ip & II pads (bf16)
        h_pool = tc.alloc_tile_pool(name="hx", bufs=5)           # 4 tensors x 5
        cf_pool = tc.alloc_tile_pool(name="cf", bufs=3)          # coeff transients
        ab_pool = tc.alloc_tile_pool(name="ab", bufs=3)          # xp_a, xp_b pads (bf16)
        hab_pool = tc.alloc_tile_pool(name="hab", bufs=4)        # ha, hb
        o_pool = tc.alloc_tile_pool(name="o", bufs=3)
        ps_pool = tc.alloc_tile_pool(name="ps", bufs=1, space="PSUM")
        psab_pool = tc.alloc_tile_pool(name="psab", bufs=2, space="PSUM")

        def mirrors(xp, eng):
            # left: cols 0:16 <- interior cols 32..17 (x[16..1]); right symmetric.
            c0 = 16 + Wc
            if eng is nc.scalar:
                eng.copy(xp[:, 0:16], xp[:, 32:16:-1])
                eng.copy(xp[:, c0 : c0 + 15], xp[:, c0 - 2 : c0 - 17 : -1])
            else:
                eng.tensor_copy(xp[:, 0:16], xp[:, 32:16:-1])
                eng.tensor_copy(xp[:, c0 : c0 + 15], xp[:, c0 - 2 : c0 - 17 : -1])

        def hscan(xp, out, dtag, pool, eng, init_bias=None):
            init = pool.tile([128, 1], f32, tag=f"init{dtag}", name=f"init{dtag}")
            nc.vector.reduce_sum(init[:], xp[:, 0:31], axis=AX)
            if init_bias is not None:
                nc.vector.tensor_scalar_add(init[:], init[:], init_bias)
            eng.tensor_tensor_scan(
                out[:], xp[:, 31 : 31 + Wc], xp[:, 0:Wc], init[:],
                op0=OP.add, op1=OP.subtract,
            )

        def vpass(psum, hsrc, j, c):
            """psum[128, CH] = normalized band-weighted column sums of hsrc tiles."""
            lo, hi = c * CH, (c + 1) * CH
            last_center = (j == 0 or hsrc[j - 1] is None) and (
                j == NT - 1 or hsrc[j + 1] is None
            )
            nc.tensor.matmul(
                psum[:], wv_sb[j][:, 0:128], hsrc[j][:, lo:hi],
                start=True, stop=last_center,
            )
            if j > 0 and hsrc[j - 1] is not None:
                nc.tensor.matmul(
                    psum[:], wv_sb[j][64:128, 128:256], hsrc[j - 1][64:128, lo:hi],
                    start=False, stop=(j == NT - 1 or hsrc[j + 1] is None),
                )
            if j < NT - 1 and hsrc[j + 1] is not None:
                nc.tensor.matmul(
                    psum[:], wv_sb[j][0:15, 256:384], hsrc[j + 1][0:15, lo:hi],
                    start=False, stop=True,
                )

        for img in range(n_img):
            xpI = [None] * NT
            hI = [None] * NT
            hp = [None] * NT
            hIp = [None] * NT
            hII = [None] * NT
            xpa = [None] * NT
            xpb = [None] * NT
            ha = [None] * NT
            hb = [None] * NT

            def stageAB(j):
                xpI[j] = xpi_pool.tile([128, PW], f32, tag="xpI", name="xpI")
                xpP = xpp_pool.tile([128, PW], f32, tag="xpP", name="xpP")
                nc.sync.dma_start(xpI[j][:, 16 : 16 + Wc], gap[img, j * 128 : (j + 1) * 128, :])
                nc.sync.dma_start(xpP[:, 16 : 16 + Wc], pap[img, j * 128 : (j + 1) * 128, :])
                mirrors(xpI[j], nc.scalar)
                mirrors(xpP, nc.scalar)
                xpIp = xpm_pool.tile([128, PW], bf16, tag="xpIp", name="xpIp")
                xpII = xpm_pool.tile([128, PW], bf16, tag="xpII", name="xpII")
                nc.vector.tensor_mul(
                    xpIp[:, 16 : 16 + Wc], xpI[j][:, 16 : 16 + Wc], xpP[:, 16 : 16 + Wc]
                )
                nc.scalar.activation(
                    xpII[:, 16 : 16 + Wc], xpI[j][:, 16 : 16 + Wc], AF.Square
                )
                mirrors(xpIp, nc.vector)
                mirrors(xpII, nc.vector)
                hI[j] = h_pool.tile([128, Wc], bf16, tag="hI", name="hI")
                hp[j] = h_pool.tile([128, Wc], bf16, tag="hp", name="hp")
                hIp[j] = h_pool.tile([128, Wc], bf16, tag="hIp", name="hIp")
                hII[j] = h_pool.tile([128, Wc], bf16, tag="hII", name="hII")
                hscan(xpI[j], hI[j], "I", cf_pool, nc.gpsimd)
                hscan(xpP, hp[j], "p", cf_pool, nc.gpsimd)
                hscan(xpIp, hIp[j], "Ip", cf_pool, nc.gpsimd)
                # +K*EPS on every hII window sum -> +EPS on normalized mean_II
                hscan(xpII, hII[j], "II", cf_pool, nc.gpsimd, init_bias=K * EPS)

            def stageCD(j):
                xpa[j] = ab_pool.tile([128, PW], bf16, tag="xpa", name="xpa")
                xpb[j] = ab_pool.tile([128, PW], bf16, tag="xpb", name="xpb")
                for c in range(NC_):
                    ps4 = ps_pool.tile([128, 4 * CH], f32, tag="ps4", name="ps4")
                    mI = ps4[:, 0:CH]
                    mp = ps4[:, CH : 2 * CH]
                    mIp = ps4[:, 2 * CH : 3 * CH]
                    mIIe = ps4[:, 3 * CH : 4 * CH]
                    vpass(mI, hI, j, c)
                    vpass(mp, hp, j, c)
                    vpass(mIp, hIp, j, c)
                    vpass(mIIe, hII, j, c)
                    # single wide evacuation of all 4 means (frees PSUM fast,
                    # enables bf16 2x DVE ops downstream)
                    mall = cf_pool.tile([128, 4 * CH], bf16, tag="mall", name="mall")
                    nc.scalar.copy(mall[:], ps4[:])
                    mIs = mall[:, 0:CH]
                    mps = mall[:, CH : 2 * CH]
                    mIps = mall[:, 2 * CH : 3 * CH]
                    mIIs = mall[:, 3 * CH : 4 * CH]
                    sq = cf_pool.tile([128, CH], bf16, tag="sq", name="sq")
                    nc.scalar.activation(sq[:], mIs, AF.Square)
                    prod = cf_pool.tile([128, CH], bf16, tag="prod", name="prod")
                    nc.vector.tensor_tensor(prod[:], mIs, mps, op=OP.mult)
                    cov = cf_pool.tile([128, CH], bf16, tag="cov", name="cov")
                    nc.vector.tensor_tensor(cov[:], mIps, prod[:], op=OP.subtract)
                    den = cf_pool.tile([128, CH], bf16, tag="den", name="den")
                    nc.vector.tensor_tensor(den[:], mIIs, sq[:], op=OP.subtract)
                    rp = cf_pool.tile([128, CH], bf16, tag="rp", name="rp")
                    nc.scalar.activation(rp[:], den[:], AF.Abs_reciprocal_sqrt)
                    u = cf_pool.tile([128, CH], bf16, tag="u", name="u")
                    nc.vector.tensor_tensor(u[:], cov[:], rp[:], op=OP.mult)
                    lo = 16 + c * CH
                    av = xpa[j][:, lo : lo + CH]
                    nc.vector.tensor_tensor(av, u[:], rp[:], op=OP.mult)
                    t = cf_pool.tile([128, CH], bf16, tag="t", name="t")
                    nc.vector.tensor_tensor(t[:], av, mIs, op=OP.mult)
                    nc.vector.tensor_tensor(
                        xpb[j][:, lo : lo + CH], mps, t[:], op=OP.subtract
                    )
                mirrors(xpa[j], nc.scalar)
                mirrors(xpb[j], nc.scalar)
                ha[j] = hab_pool.tile([128, Wc], bf16, tag="ha", name="ha")
                hb[j] = hab_pool.tile([128, Wc], bf16, tag="hb", name="hb")
                hscan(xpa[j], ha[j], "a", cf_pool, nc.gpsimd)
                hscan(xpb[j], hb[j], "b", cf_pool, nc.gpsimd)

            def stageF(j):
                for c in range(NC_):
                    ma = psab_pool.tile([128, CH], f32, tag="psa", name="psa")
                    mb = psab_pool.tile([128, CH], f32, tag="psb", name="psb")
                    vpass(ma, ha, j, c)
                    vpass(mb, hb, j, c)
                    o1 = o_pool.tile([128, CH], f32, tag="o1", name="o1")
                    nc.vector.tensor_tensor(
                        o1[:], ma[:], xpI[j][:, 16 + c * CH : 16 + (c + 1) * CH],
                        op=OP.mult,
                    )
                    o2 = o_pool.tile([128, CH], f32, tag="o2", name="o2")
                    nc.vector.tensor_tensor(o2[:], o1[:], mb[:], op=OP.add)
                    nc.sync.dma_start(
                        oap[img, j * 128 : (j + 1) * 128, c * CH : (c + 1) * CH], o2[:]
                    )

            # software-pipelined emission: AB leads CD by 3 tiles, F lags CD by 1
            LEAD = 3
            for jj in range(min(LEAD, NT)):
                stageAB(jj)
            for j in range(NT):
                stageCD(j)
                if j >= 1:
                    stageF(j - 1)
                if j + LEAD < NT:
                    stageAB(j + LEAD)
            stageF(NT - 1)

        for _pool in (psab_pool, ps_pool, o_pool, hab_pool, ab_pool,
                      cf_pool, h_pool, xpm_pool, xpp_pool, xpi_pool, wpool):
            _pool.release()

    nc.compile()
    return nc


def _get_nc(n_img, Hc, Wc):
    key = (n_img, Hc, Wc)
    if key not in _CACHE:
        _CACHE[key] = build_nc(n_img, Hc, Wc)
    return _CACHE[key]


def kernel(guide, input_map):
    from concourse.bass_utils import run_bass_kernel_spmd

    B, C, Hc, Wc = guide.shape
    n_cores = 8
    n_img = B // n_cores
    g = np.ascontiguousarray(guide.reshape(B, Hc, Wc), dtype=np.float32)
    p = np.ascontiguousarray(input_map.reshape(B, Hc, Wc), dtype=np.float32)
    wv = _build_band_weights(Hc, Hc // 128)
    nc = _get_nc(n_img, Hc, Wc)
    in_maps = [
        {
            "guide": g[i * n_img : (i + 1) * n_img],
            "input_map": p[i * n_img : (i + 1) * n_img],
            "wv": wv,
        }
        for i in range(n_cores)
    ]
    res = run_bass_kernel_spmd(nc, in_maps, core_ids=list(range(n_cores)))
    out = np.concatenate([res.results[i]["out"] for i in range(n_cores)], axis=0)
    return out.reshape(B, C, Hc, Wc).astype(np.float32)


# revision 24
# speedup vs baseline: 1.1155x; 1.1155x over previous
"""GuidedFilter (r=15, eps=0.5) Trainium2 Bass kernel.

Full inputs: guide, input_map [16,1,1024,1024] f32. Data-parallel over 8
NeuronCores (2 images/core). Per image:
  box(x) = Vpass(Hpass(x)) with 31-tap window sums, reflect padding.
  - H direction (free axis): tensor_tensor_scan per row-tile, split across
    DVE and Pool (gpsimd) engines to balance load.
  - V direction (partition axis): PE band matmuls with constant bf16
    weights (reflect folded into band blocks, 1/961 normalization folded
    into the weights), fp32 PSUM accumulate.
  - eps is folded into the II H-scan init (+31*EPS offsets every window
    sum, which the normalized V-pass turns into +EPS on mean_II).
  - a = cov * r, r = 1/(var+eps) via Act-engine Abs_reciprocal_sqrt
    squared on DVE; elementwise chain mostly bf16.
"""

import numpy as np
import ml_dtypes

R = 15
K = 2 * R + 1  # 31
EPS = 0.5
NORM = 1.0 / (K * K)  # 1/961

_CACHE = {}


def _build_band_weights(Hc, NT):
    """Wf[k, m] = NORM * weight of input row k in output row m's reflect window."""
    Wf = np.zeros((Hc, Hc), np.float32)
    for m in range(Hc):
        for t in range(m - R, m + R + 1):
            k = t
            if k < 0:
                k = -k
            if k > Hc - 1:
                k = 2 * (Hc - 1) - k
            Wf[k, m] += 1.0
    Wf *= NORM
    # Pack per out-tile j into [128, 3*128]:
    #   cols 0:128   = center block  (in-tile j)
    #   cols 128:256 = top edge      (in-tile j-1 rows 113:128 -> rows 64:128 window)
    #   cols 256:384 = bottom edge   (in-tile j+1 rows 0:15)
    wv = np.zeros((NT, 128, 384), np.float32)
    for j in range(NT):
        r0 = j * 128
        wv[j, :, 0:128] = Wf[r0 : r0 + 128, r0 : r0 + 128]
        if j > 0:
            wv[j, 64:128, 128:256] = Wf[r0 - 64 : r0, r0 : r0 + 128]
        if j < NT - 1:
            wv[j, 0:15, 256:384] = Wf[r0 + 128 : r0 + 143, r0 : r0 + 128]
    return wv.astype(ml_dtypes.bfloat16)


def build_nc(n_img, Hc, Wc):
    """Build the Bass module for one core processing n_img images of [Hc, Wc]."""
    import concourse.bass as bass
    import concourse.tile as tile
    from concourse import bacc, mybir

    P = 128
    NT = Hc // P
    PW = Wc + 32          # padded width; interior at cols 16..16+Wc
    CH = min(512, Wc)     # psum chunk width
    NC_ = Wc // CH        # chunks per tile
    f32 = mybir.dt.float32
    bf16 = mybir.dt.bfloat16
    AX = mybir.AxisListType.X
    OP = mybir.AluOpType
    AF = mybir.ActivationFunctionType

    nc = bacc.Bacc("TRN2", target_bir_lowering=False, debug=False)
    g_dram = nc.dram_tensor("guide", [n_img, Hc, Wc], f32, kind="ExternalInput")
    p_dram = nc.dram_tensor("input_map", [n_img, Hc, Wc], f32, kind="ExternalInput")
    wv_dram = nc.dram_tensor("wv", [NT, 128, 384], bf16, kind="ExternalInput")
    o_dram = nc.dram_tensor("out", [n_img, Hc, Wc], f32, kind="ExternalOutput")
    gap, pap, wap, oap = g_dram.ap(), p_dram.ap(), wv_dram.ap(), o_dram.ap()

    with tile.TileContext(nc) as tc:
        wpool = tc.alloc_tile_pool(name="wv", bufs=1)
        wv_sb = []
        for j in range(NT):
            wt = wpool.tile([128, 384], bf16, tag=f"wv{j}", name=f"wv{j}")
            nc.sync.dma_start(wt[:], wap[j])
            wv_sb.append(wt)

        xpi_pool = tc.alloc_tile_pool(name="xpi", bufs=NT + 1)   # guide, image-long
        xpp_pool = tc.alloc_tile_pool(name="xpp", bufs=3)
        xpm_pool = tc.alloc_tile_pool(name="xpm", bufs=2)        # Ip & II pads (bf16)
        h_pool = tc.alloc_tile_pool(name="hx", bufs=5)           # 4 tensors x 5
        cf_pool = tc.alloc_tile_pool(name="cf", bufs=3)          # coeff transients
        ab_pool = tc.alloc_tile_pool(name="ab", bufs=3)          # xp_a, xp_b pads (bf16)
        hab_pool = tc.alloc_tile_pool(name="hab", bufs=4)        # ha, hb
        o_pool = tc.alloc_tile_pool(name="o", bufs=3)
        ps_pool = tc.alloc_tile_pool(name="ps", bufs=1, space="PSUM")
        psab_pool = tc.alloc_tile_pool(name="psab", bufs=2, space="PSUM")

        def mirrors(xp, eng):
            # left: cols 0:16 <- interior cols 32..17 (x[16..1]); right symmetric.
            c0 = 16 + Wc
            if eng is nc.scalar:
                eng.copy(xp[:, 0:16], xp[:, 32:16:-1])
                eng.copy(xp[:, c0 : c0 + 15], xp[:, c0 - 2 : c0 - 17 : -1])
            else:
                eng.tensor_copy(xp[:, 0:16], xp[:, 32:16:-1])
                eng.tensor_copy(xp[:, c0 : c0 + 15], xp[:, c0 - 2 : c0 - 17 : -1])

        def hscan(xp, out, dtag, pool, eng, init_bias=None):
            init = pool.tile([128, 1], f32, tag=f"init{dtag}", name=f"init{dtag}")
            nc.vector.reduce_sum(init[:], xp[:, 0:31], axis=AX)
            if init_bias is not None:
                nc.vector.tensor_scalar_add(init[:], init[:], init_bias)
            eng.tensor_tensor_scan(
                out[:], xp[:, 31 : 31 + Wc], xp[:, 0:Wc], init[:],
                op0=OP.add, op1=OP.subtract,
            )

        def vpass(psum, hsrc, j, c):
            """psum[128, CH] = normalized band-weighted column sums of hsrc tiles."""
            lo, hi = c * CH, (c + 1) * CH
            last_center = (j == 0 or hsrc[j - 1] is None) and (
                j == NT - 1 or hsrc[j + 1] is None
            )
            nc.tensor.matmul(
                psum[:], wv_sb[j][:, 0:128], hsrc[j][:, lo:hi],
                start=True, stop=last_center,
            )
            if j > 0 and hsrc[j - 1] is not None:
                nc.tensor.matmul(
                    psum[:], wv_sb[j][64:128, 128:256], hsrc[j - 1][64:128, lo:hi],
                    start=False, stop=(j == NT - 1 or hsrc[j + 1] is None),
                )
            if j < NT - 1 and hsrc[j + 1] is not None:
                nc.tensor.matmul(
                    psum[:], wv_sb[j][0:15, 256:384], hsrc[j + 1][0:15, lo:hi],
                    start=False, stop=True,
                )

        for img in range(n_img):
            xpI = [None] * NT
            hI = [None] * NT
            hp = [None] * NT
            hIp = [None] * NT
            hII = [None] * NT
            xpa = [None] * NT
            xpb = [None] * NT
            ha = [None] * NT
            hb = [None] * NT

            def stageAB(j):
                xpI[j] = xpi_pool.tile([128, PW], f32, tag="xpI", name="xpI")
                xpP = xpp_pool.tile([128, PW], f32, tag="xpP", name="xpP")
                nc.sync.dma_start(xpI[j][:, 16 : 16 + Wc], gap[img, j * 128 : (j + 1) * 128, :])
                nc.sync.dma_start(xpP[:, 16 : 16 + Wc], pap[img, j * 128 : (j + 1) * 128, :])
                mirrors(xpI[j], nc.scalar)
                mirrors(xpP, nc.scalar)
                xpIp = xpm_pool.tile([128, PW], bf16, tag="xpIp", name="xpIp")
                xpII = xpm_pool.tile([128, PW], bf16, tag="xpII", name="xpII")
                nc.vector.tensor_mul(
                    xpIp[:, 16 : 16 + Wc], xpI[j][:, 16 : 16 + Wc], xpP[:, 16 : 16 + Wc]
                )
                nc.scalar.activation(
                    xpII[:, 16 : 16 + Wc], xpI[j][:, 16 : 16 + Wc], AF.Square
                )
                mirrors(xpIp, nc.vector)
                mirrors(xpII, nc.vector)
                hI[j] = h_pool.tile([128, Wc], bf16, tag="hI", name="hI")
                hp[j] = h_pool.tile([128, Wc], bf16, tag="hp", name="hp")
                hIp[j] = h_pool.tile([128, Wc], bf16, tag="hIp", name="hIp")
                hII[j] = h_pool.tile([128, Wc], bf16, tag="hII", name="hII")
                hscan(xpI[j], hI[j], "I", cf_pool, nc.gpsimd)
                hscan(xpP, hp[j], "p", cf_pool, nc.gpsimd)
                hscan(xpIp, hIp[j], "Ip", cf_pool, nc.gpsimd)
                # +K*EPS on every hII window sum -> +EPS on normalized mean_II
                hscan(xpII, hII[j], "II", cf_pool, nc.gpsimd, init_bias=K * EPS)

            def stageCD(j):
                xpa[j] = ab_pool.tile([128, PW], bf16, tag="xpa", name="xpa")
                xpb[j] = ab_pool.tile([128, PW], bf16, tag="xpb", name="xpb")
                for c in range(NC_):
                    psAB = ps_pool.tile([128, 2 * CH], f32, tag="psAB", name="psAB")
                    psCD = ps_pool.tile([128, 2 * CH], f32, tag="psCD", name="psCD")
                    mI = psAB[:, 0:CH]
                    mp = psAB[:, CH : 2 * CH]
                    mIp = psCD[:, 0:CH]
                    mIIe = psCD[:, CH : 2 * CH]
                    vpass(mI, hI, j, c)
                    vpass(mp, hp, j, c)
                    vpass(mIp, hIp, j, c)
                    vpass(mIIe, hII, j, c)
                    # paired evacuation of the 4 means (frees PSUM fast,
                    # enables bf16 2x DVE ops downstream)
                    mab = cf_pool.tile([128, 2 * CH], bf16, tag="mab", name="mab")
                    nc.scalar.copy(mab[:], psAB[:])
                    mcd = cf_pool.tile([128, 2 * CH], bf16, tag="mcd", name="mcd")
                    nc.scalar.copy(mcd[:], psCD[:])
                    mIs = mab[:, 0:CH]
                    mps = mab[:, CH : 2 * CH]
                    mIps = mcd[:, 0:CH]
                    mIIs = mcd[:, CH : 2 * CH]
                    sq = cf_pool.tile([128, CH], bf16, tag="sq", name="sq")
                    nc.scalar.activation(sq[:], mIs, AF.Square)
                    prod = cf_pool.tile([128, CH], bf16, tag="prod", name="prod")
                    nc.vector.tensor_tensor(prod[:], mIs, mps, op=OP.mult)
                    cov = cf_pool.tile([128, CH], bf16, tag="cov", name="cov")
                    nc.vector.tensor_tensor(cov[:], mIps, prod[:], op=OP.subtract)
                    den = cf_pool.tile([128, CH], bf16, tag="den", name="den")
                    nc.vector.tensor_tensor(den[:], mIIs, sq[:], op=OP.subtract)
                    rp = cf_pool.tile([128, CH], bf16, tag="rp", name="rp")
                    nc.scalar.activation(rp[:], den[:], AF.Abs_reciprocal_sqrt)
                    u = cf_pool.tile([128, CH], bf16, tag="u", name="u")
                    nc.vector.tensor_tensor(u[:], cov[:], rp[:], op=OP.mult)
                    lo = 16 + c * CH
                    av = xpa[j][:, lo : lo + CH]
                    nc.vector.tensor_tensor(av, u[:], rp[:], op=OP.mult)
                    t = cf_pool.tile([128, CH], bf16, tag="t", name="t")
                    nc.vector.tensor_tensor(t[:], av, mIs, op=OP.mult)
                    nc.vector.tensor_tensor(
                        xpb[j][:, lo : lo + CH], mps, t[:], op=OP.subtract
                    )
                mirrors(xpa[j], nc.scalar)
                mirrors(xpb[j], nc.scalar)
                ha[j] = hab_pool.tile([128, Wc], bf16, tag="ha", name="ha")
                hb[j] = hab_pool.tile([128, Wc], bf16, tag="hb", name="hb")
                hscan(xpa[j], ha[j], "a", cf_pool, nc.gpsimd)
                hscan(xpb[j], hb[j], "b", cf_pool, nc.gpsimd)

            def stageF(j):
                for c in range(NC_):
                    ma = psab_pool.tile([128, CH], f32, tag="psa", name="psa")
                    mb = psab_pool.tile([128, CH], f32, tag="psb", name="psb")
                    vpass(ma, ha, j, c)
                    vpass(mb, hb, j, c)
                    o1 = o_pool.tile([128, CH], f32, tag="o1", name="o1")
                    nc.vector.tensor_tensor(
                        o1[:], ma[:], xpI[j][:, 16 + c * CH : 16 + (c + 1) * CH],
                        op=OP.mult,
                    )
                    o2 = o_pool.tile([128, CH], f32, tag="o2", name="o2")
                    nc.vector.tensor_tensor(o2[:], o1[:], mb[:], op=OP.add)
                    nc.sync.dma_start(
                        oap[img, j * 128 : (j + 1) * 128, c * CH : (c + 1) * CH], o2[:]
                    )

            # software-pipelined emission: AB leads CD by 3 tiles, F lags CD by 1
            LEAD = 3
            for jj in range(min(LEAD, NT)):
                stageAB(jj)
            for j in range(NT):
                stageCD(j)
                if j >= 1:
                    stageF(j - 1)
                if j + LEAD < NT:
                    stageAB(j + LEAD)
            stageF(NT - 1)

        for _pool in (psab_pool, ps_pool, o_pool, hab_pool, ab_pool,
                      cf_pool, h_pool, xpm_pool, xpp_pool, xpi_pool, wpool):
            _pool.release()

    nc.compile()
    return nc


def _get_nc(n_img, Hc, Wc):
    key = (n_img, Hc, Wc)
    if key not in _CACHE:
        _CACHE[key] = build_nc(n_img, Hc, Wc)
    return _CACHE[key]


def kernel(guide, input_map):
    from concourse.bass_utils import run_bass_kernel_spmd

    B, C, Hc, Wc = guide.shape
    n_cores = 8
    n_img = B // n_cores
    g = np.ascontiguousarray(guide.reshape(B, Hc, Wc), dtype=np.float32)
    p = np.ascontiguousarray(input_map.reshape(B, Hc, Wc), dtype=np.float32)
    wv = _build_band_weights(Hc, Hc // 128)
    nc = _get_nc(n_img, Hc, Wc)
    in_maps = [
        {
            "guide": g[i * n_img : (i + 1) * n_img],
            "input_map": p[i * n_img : (i + 1) * n_img],
            "wv": wv,
        }
        for i in range(n_cores)
    ]
    res = run_bass_kernel_spmd(nc, in_maps, core_ids=list(range(n_cores)))
    out = np.concatenate([res.results[i]["out"] for i in range(n_cores)], axis=0)
    return out.reshape(B, C, Hc, Wc).astype(np.float32)


# revision 26
# speedup vs baseline: 1.1226x; 1.0064x over previous
"""GuidedFilter (r=15, eps=0.5) Trainium2 Bass kernel.

Full inputs: guide, input_map [16,1,1024,1024] f32. Data-parallel over 8
NeuronCores (2 images/core). Per image:
  box(x) = Vpass(Hpass(x)) with 31-tap window sums, reflect padding.
  - H direction (free axis): tensor_tensor_scan per row-tile, split across
    DVE and Pool (gpsimd) engines to balance load.
  - V direction (partition axis): PE band matmuls with constant bf16
    weights (reflect folded into band blocks, 1/961 normalization folded
    into the weights), fp32 PSUM accumulate.
  - eps is folded into the II H-scan init (+31*EPS offsets every window
    sum, which the normalized V-pass turns into +EPS on mean_II).
  - a = cov * r, r = 1/(var+eps) via Act-engine Abs_reciprocal_sqrt
    squared on DVE; elementwise chain mostly bf16.
"""

import numpy as np
import ml_dtypes

R = 15
K = 2 * R + 1  # 31
EPS = 0.5
NORM = 1.0 / (K * K)  # 1/961

_CACHE = {}


def _build_band_weights(Hc, NT):
    """Wf[k, m] = NORM * weight of input row k in output row m's reflect window."""
    Wf = np.zeros((Hc, Hc), np.float32)
    for m in range(Hc):
        for t in range(m - R, m + R + 1):
            k = t
            if k < 0:
                k = -k
            if k > Hc - 1:
                k = 2 * (Hc - 1) - k
            Wf[k, m] += 1.0
    Wf *= NORM
    # Pack per out-tile j into [128, 3*128]:
    #   cols 0:128   = center block  (in-tile j)
    #   cols 128:256 = top edge      (in-tile j-1 rows 113:128 -> rows 64:128 window)
    #   cols 256:384 = bottom edge   (in-tile j+1 rows 0:15)
    wv = np.zeros((NT, 128, 384), np.float32)
    for j in range(NT):
        r0 = j * 128
        wv[j, :, 0:128] = Wf[r0 : r0 + 128, r0 : r0 + 128]
        if j > 0:
            wv[j, 64:128, 128:256] = Wf[r0 - 64 : r0, r0 : r0 + 128]
        if j < NT - 1:
            wv[j, 0:15, 256:384] = Wf[r0 + 128 : r0 + 143, r0 : r0 + 128]
    return wv.astype(ml_dtypes.bfloat16)


def build_nc(n_img, Hc, Wc):
    """Build the Bass module for one core processing n_img images of [Hc, Wc]."""
    import concourse.bass as bass
    import concourse.tile as tile
    from concourse import bacc, mybir

    P = 128
    NT = Hc // P
    PW = Wc + 32          # padded width; interior at cols 16..16+Wc
    CH = min(512, Wc)     # psum chunk width
    NC_ = Wc // CH        # chunks per tile
    f32 = mybir.dt.float32
    bf16 = mybir.dt.bfloat16
    AX = mybir.AxisListType.X
    OP = mybir.AluOpType
    AF = mybir.ActivationFunctionType

    nc = bacc.Bacc("TRN2", target_bir_lowering=False, debug=False)
    g_dram = nc.dram_tensor("guide", [n_img, Hc, Wc], f32, kind="ExternalInput")
    p_dram = nc.dram_tensor("input_map", [n_img, Hc, Wc], f32, kind="ExternalInput")
    wv_dram = nc.dram_tensor("wv", [NT, 128, 384], bf16, kind="ExternalInput")
    o_dram = nc.dram_tensor("out", [n_img, Hc, Wc], f32, kind="ExternalOutput")
    gap, pap, wap, oap = g_dram.ap(), p_dram.ap(), wv_dram.ap(), o_dram.ap()

    with tile.TileContext(nc) as tc:
        wpool = tc.alloc_tile_pool(name="wv", bufs=1)
        wv_sb = []
        for j in range(NT):
            wt = wpool.tile([128, 384], bf16, tag=f"wv{j}", name=f"wv{j}")
            nc.sync.dma_start(wt[:], wap[j])
            wv_sb.append(wt)

        xpi_pool = tc.alloc_tile_pool(name="xpi", bufs=NT + 1)   # guide, image-long
        xpp_pool = tc.alloc_tile_pool(name="xpp", bufs=3)
        xpm_pool = tc.alloc_tile_pool(name="xpm", bufs=2)        # Ip & II pads (bf16)
        h_pool = tc.alloc_tile_pool(name="hx", bufs=5)           # 4 tensors x 5
        cf_pool = tc.alloc_tile_pool(name="cf", bufs=3)          # coeff transients
        ab_pool = tc.alloc_tile_pool(name="ab", bufs=3)          # xp_a, xp_b pads (bf16)
        hab_pool = tc.alloc_tile_pool(name="hab", bufs=4)        # ha, hb
        o_pool = tc.alloc_tile_pool(name="o", bufs=3)
        ps_pool = tc.alloc_tile_pool(name="ps", bufs=1, space="PSUM")
        psab_pool = tc.alloc_tile_pool(name="psab", bufs=2, space="PSUM")

        def mirrors(xp, eng):
            # left: cols 0:16 <- interior cols 32..17 (x[16..1]); right symmetric.
            c0 = 16 + Wc
            if eng is nc.scalar:
                eng.copy(xp[:, 0:16], xp[:, 32:16:-1])
                eng.copy(xp[:, c0 : c0 + 15], xp[:, c0 - 2 : c0 - 17 : -1])
            else:
                eng.tensor_copy(xp[:, 0:16], xp[:, 32:16:-1])
                eng.tensor_copy(xp[:, c0 : c0 + 15], xp[:, c0 - 2 : c0 - 17 : -1])

        def hscan(xp, out, dtag, pool, eng, init_bias=None):
            init = pool.tile([128, 1], f32, tag=f"init{dtag}", name=f"init{dtag}")
            nc.vector.reduce_sum(init[:], xp[:, 0:31], axis=AX)
            if init_bias is not None:
                nc.vector.tensor_scalar_add(init[:], init[:], init_bias)
            eng.tensor_tensor_scan(
                out[:], xp[:, 31 : 31 + Wc], xp[:, 0:Wc], init[:],
                op0=OP.add, op1=OP.subtract,
            )

        def vpass(psum, hsrc, j, c):
            """psum[128, CH] = normalized band-weighted column sums of hsrc tiles."""
            lo, hi = c * CH, (c + 1) * CH
            last_center = (j == 0 or hsrc[j - 1] is None) and (
                j == NT - 1 or hsrc[j + 1] is None
            )
            nc.tensor.matmul(
                psum[:], wv_sb[j][:, 0:128], hsrc[j][:, lo:hi],
                start=True, stop=last_center,
            )
            if j > 0 and hsrc[j - 1] is not None:
                nc.tensor.matmul(
                    psum[:], wv_sb[j][64:128, 128:256], hsrc[j - 1][64:128, lo:hi],
                    start=False, stop=(j == NT - 1 or hsrc[j + 1] is None),
                )
            if j < NT - 1 and hsrc[j + 1] is not None:
                nc.tensor.matmul(
                    psum[:], wv_sb[j][0:15, 256:384], hsrc[j + 1][0:15, lo:hi],
                    start=False, stop=True,
                )

        for img in range(n_img):
            xpI = [None] * NT
            hI = [None] * NT
            hp = [None] * NT
            hIp = [None] * NT
            hII = [None] * NT
            xpa = [None] * NT
            xpb = [None] * NT
            ha = [None] * NT
            hb = [None] * NT

            def stageAB(j):
                xpI[j] = xpi_pool.tile([128, PW], f32, tag="xpI", name="xpI")
                xpP = xpp_pool.tile([128, PW], f32, tag="xpP", name="xpP")
                nc.sync.dma_start(xpI[j][:, 16 : 16 + Wc], gap[img, j * 128 : (j + 1) * 128, :])
                nc.sync.dma_start(xpP[:, 16 : 16 + Wc], pap[img, j * 128 : (j + 1) * 128, :])
                mirrors(xpI[j], nc.scalar)
                mirrors(xpP, nc.scalar)
                xpIp = xpm_pool.tile([128, PW], bf16, tag="xpIp", name="xpIp")
                xpII = xpm_pool.tile([128, PW], bf16, tag="xpII", name="xpII")
                nc.vector.tensor_mul(
                    xpIp[:, 16 : 16 + Wc], xpI[j][:, 16 : 16 + Wc], xpP[:, 16 : 16 + Wc]
                )
                nc.scalar.activation(
                    xpII[:, 16 : 16 + Wc], xpI[j][:, 16 : 16 + Wc], AF.Square
                )
                mirrors(xpIp, nc.vector)
                mirrors(xpII, nc.vector)
                hI[j] = h_pool.tile([128, Wc], bf16, tag="hI", name="hI")
                hp[j] = h_pool.tile([128, Wc], bf16, tag="hp", name="hp")
                hIp[j] = h_pool.tile([128, Wc], bf16, tag="hIp", name="hIp")
                hII[j] = h_pool.tile([128, Wc], bf16, tag="hII", name="hII")
                hscan(xpI[j], hI[j], "I", cf_pool, nc.gpsimd)
                hscan(xpP, hp[j], "p", cf_pool, nc.gpsimd)
                hscan(xpIp, hIp[j], "Ip", cf_pool, nc.gpsimd)
                # +K*EPS on every hII window sum -> +EPS on normalized mean_II
                hscan(xpII, hII[j], "II", cf_pool, nc.gpsimd, init_bias=K * EPS)

            def stageCD(j):
                xpa[j] = ab_pool.tile([128, PW], bf16, tag="xpa", name="xpa")
                xpb[j] = ab_pool.tile([128, PW], bf16, tag="xpb", name="xpb")
                st = []
                for c in range(NC_):
                    psAB = ps_pool.tile([128, 2 * CH], f32, tag="psAB", name="psAB")
                    psCD = ps_pool.tile([128, 2 * CH], f32, tag="psCD", name="psCD")
                    vpass(psAB[:, 0:CH], hI, j, c)
                    vpass(psAB[:, CH : 2 * CH], hp, j, c)
                    vpass(psCD[:, 0:CH], hIp, j, c)
                    vpass(psCD[:, CH : 2 * CH], hII, j, c)
                    # paired evacuation of the 4 means (frees PSUM fast,
                    # enables bf16 2x DVE ops downstream)
                    mab = cf_pool.tile([128, 2 * CH], bf16, tag="mab", name="mab")
                    nc.scalar.copy(mab[:], psAB[:])
                    mcd = cf_pool.tile([128, 2 * CH], bf16, tag="mcd", name="mcd")
                    nc.scalar.copy(mcd[:], psCD[:])
                    st.append((mab, mcd))
                sqs, covs, dens, rps = [], [], [], []
                for c in range(NC_):
                    mab, mcd = st[c]
                    sq = cf_pool.tile([128, CH], bf16, tag="sq", name="sq")
                    nc.scalar.activation(sq[:], mab[:, 0:CH], AF.Square)
                    sqs.append(sq)
                    prod = cf_pool.tile([128, CH], bf16, tag="prod", name="prod")
                    nc.vector.tensor_tensor(prod[:], mab[:, 0:CH], mab[:, CH:], op=OP.mult)
                    cov = cf_pool.tile([128, CH], bf16, tag="cov", name="cov")
                    nc.vector.tensor_tensor(cov[:], mcd[:, 0:CH], prod[:], op=OP.subtract)
                    covs.append(cov)
                for c in range(NC_):
                    mab, mcd = st[c]
                    den = cf_pool.tile([128, CH], bf16, tag="den", name="den")
                    nc.vector.tensor_tensor(den[:], mcd[:, CH:], sqs[c][:], op=OP.subtract)
                    rp = cf_pool.tile([128, CH], bf16, tag="rp", name="rp")
                    nc.scalar.activation(rp[:], den[:], AF.Abs_reciprocal_sqrt)
                    rps.append(rp)
                for c in range(NC_):
                    mab, mcd = st[c]
                    rp = rps[c]
                    u = cf_pool.tile([128, CH], bf16, tag="u", name="u")
                    nc.vector.tensor_tensor(u[:], covs[c][:], rp[:], op=OP.mult)
                    lo = 16 + c * CH
                    av = xpa[j][:, lo : lo + CH]
                    nc.vector.tensor_tensor(av, u[:], rp[:], op=OP.mult)
                    t = cf_pool.tile([128, CH], bf16, tag="t", name="t")
                    nc.vector.tensor_tensor(t[:], av, mab[:, 0:CH], op=OP.mult)
                    nc.vector.tensor_tensor(
                        xpb[j][:, lo : lo + CH], mab[:, CH:], t[:], op=OP.subtract
                    )
                mirrors(xpa[j], nc.scalar)
                mirrors(xpb[j], nc.scalar)
                ha[j] = hab_pool.tile([128, Wc], bf16, tag="ha", name="ha")
                hb[j] = hab_pool.tile([128, Wc], bf16, tag="hb", name="hb")
                hscan(xpa[j], ha[j], "a", cf_pool, nc.gpsimd)
                hscan(xpb[j], hb[j], "b", cf_pool, nc.gpsimd)

            def stageF(j):
                ms = []
                for c in range(NC_):
                    ma = psab_pool.tile([128, CH], f32, tag="psa", name="psa")
                    mb = psab_pool.tile([128, CH], f32, tag="psb", name="psb")
                    vpass(ma, ha, j, c)
                    vpass(mb, hb, j, c)
                    ms.append((ma, mb))
                for c in range(NC_):
                    ma, mb = ms[c]
                    o1 = o_pool.tile([128, CH], f32, tag="o1", name="o1")
                    nc.vector.tensor_tensor(
                        o1[:], ma[:], xpI[j][:, 16 + c * CH : 16 + (c + 1) * CH],
                        op=OP.mult,
                    )
                    o2 = o_pool.tile([128, CH], f32, tag="o2", name="o2")
                    nc.vector.tensor_tensor(o2[:], o1[:], mb[:], op=OP.add)
                    nc.sync.dma_start(
                        oap[img, j * 128 : (j + 1) * 128, c * CH : (c + 1) * CH], o2[:]
                    )

            # software-pipelined emission: AB leads CD by 3 tiles, F lags CD by 1
            LEAD = 3
            for jj in range(min(LEAD, NT)):
                stageAB(jj)
            for j in range(NT):
                stageCD(j)
                if j >= 1:
                    stageF(j - 1)
                if j + LEAD < NT:
                    stageAB(j + LEAD)
            stageF(NT - 1)

        for _pool in (psab_pool, ps_pool, o_pool, hab_pool, ab_pool,
                      cf_pool, h_pool, xpm_pool, xpp_pool, xpi_pool, wpool):
            _pool.release()

    nc.compile()
    return nc


def _get_nc(n_img, Hc, Wc):
    key = (n_img, Hc, Wc)
    if key not in _CACHE:
        _CACHE[key] = build_nc(n_img, Hc, Wc)
    return _CACHE[key]


def kernel(guide, input_map):
    from concourse.bass_utils import run_bass_kernel_spmd

    B, C, Hc, Wc = guide.shape
    n_cores = 8
    n_img = B // n_cores
    g = np.ascontiguousarray(guide.reshape(B, Hc, Wc), dtype=np.float32)
    p = np.ascontiguousarray(input_map.reshape(B, Hc, Wc), dtype=np.float32)
    wv = _build_band_weights(Hc, Hc // 128)
    nc = _get_nc(n_img, Hc, Wc)
    in_maps = [
        {
            "guide": g[i * n_img : (i + 1) * n_img],
            "input_map": p[i * n_img : (i + 1) * n_img],
            "wv": wv,
        }
        for i in range(n_cores)
    ]
    res = run_bass_kernel_spmd(nc, in_maps, core_ids=list(range(n_cores)))
    out = np.concatenate([res.results[i]["out"] for i in range(n_cores)], axis=0)
    return out.reshape(B, C, Hc, Wc).astype(np.float32)
